# revision 34
# baseline (speedup 1.0000x reference)
"""Trainium2 Bass kernel for nn_Decoder1 (linear -> BatchNorm1d -> multistep LIF).

Reference computation (T=4, B=32, N=1024, C=256):
  y[tb,o,n]   = sum_c x[tb,n,c] * W[o,c]                      (TB=128 slices)
  z           = BN(y) over (tb, n) per channel o (training stats, eps=1e-5)
  LIF over t  : v' = (v + z_t)/2 ; s = (v' >= 1) ; v' *= (1-s)
  out[tb,n',c'] = spikes[tb].reshape(C,N).T   (row-major reinterpretation)

Sharding: data-parallel over B (batch) -> 4 b-values x 4 timesteps = 16
(N,C) slices per core. BN statistics via a tiny AllReduce of per-core
(sum, sumsq).

Matmul: single-term f32r (tolerance is rel-l2 2e-2 on binary spikes; f32r
rounding flips only a tiny number of near-threshold spikes).

Engine placement: PE transposes x + matmuls; ACT casts x^T f32->f32r and
applies BN in phase 2; DVE does bn_stats + LIF updates; Pool (gpsimd)
drains y PSUM->SBUF and shares LIF spike/reset work with DVE.

Layout trick: x rows are loaded in a permuted order (n = 4q+r -> column
j = 256r+q of the transposed moving operand) so the final spike tiles DMA
out to the (TB, C, N)->(TB, N, C) reinterpreted output with contiguous 1KB
runs on the DRAM side (no strided SBUF reads).
"""

import numpy as np
from contextlib import ExitStack

import concourse.bass as bass
import concourse.mybir as mybir
import concourse.tile as tile

F32 = mybir.dt.float32
F32R = mybir.dt.float32r
Alu = mybir.AluOpType
ActF = mybir.ActivationFunctionType

N_CORES = 8
T, B, N, C = 4, 32, 1024, 256
B_LOC = B // N_CORES            # 4 batch entries per core
SL = T * B_LOC                  # 16 (N,C) slices per core; sl = bl*4 + t
P = 128
NS_CORE = float(SL * N)         # BN samples per channel per core
NS_TOT = float(T * B * N)       # BN samples per channel globally
BN_EPS = 1e-5

_ctr = [0]
SINGLE = False   # test-only: skip the AllReduce (for single-core TimelineSim)
REPEATS = 1      # test-only: replicate the whole pipeline body for slope timing
ABLATE = set()   # test-only (sim): {"mm", "transpose", "ycopy", "stats", "p2", "dma_in", "dma_out"}
BUFS = {"natp": 2, "xtsb": 4, "yps": 4, "lifp": 3, "vpool": 2, "xtps": 2}


def _legalize_waits(nc, limit=1):
    """This walrus accepts very few semaphore waits per instruction (PE
    matmul: 1).  Hoist excess waits onto same-engine NoOps inserted just
    before the overloaded instruction (same engine => in-order => identical
    semantics)."""
    for f in nc.m.functions:
        for bb in f.blocks:
            new, dirty = [], False
            for ins in bb.instructions:
                si = ins.sync_info
                if si is not None and len(si.on_wait) > limit:
                    waits = list(si.on_wait)
                    for w in waits[:-limit]:
                        _ctr[0] += 1
                        no = mybir.InstNoOp(name=f"zwaitnop-{_ctr[0]}", ins=[], outs=[])
                        no.engine = ins.engine
                        no.sync_info = mybir.SyncInfo(on_wait=[w], on_update=[])
                        new.append(no)
                    ins.sync_info = mybir.SyncInfo(
                        on_wait=waits[-limit:], on_update=list(si.on_update)
                    )
                    dirty = True
                new.append(ins)
            if dirty:
                bb.instructions = new


def _build():
    nc = bass.Bass(num_devices=N_CORES)
    x_in = nc.declare_dram_parameter("x", [SL, N, C], F32R, isOutput=False)
    id_in = nc.declare_dram_parameter("ident", [P, P], F32R, isOutput=False)
    w_in = nc.declare_dram_parameter("W", [C, C], F32R, isOutput=False)
    g_in = nc.declare_dram_parameter("gamma", [C], F32, isOutput=False)
    b_in = nc.declare_dram_parameter("beta", [C], F32, isOutput=False)
    out = nc.declare_dram_parameter("out", [SL, N, C], F32, isOutput=True)

    # x rows n = h*512 + q*4 + r loaded so partition=q, free=(r,h,c):
    # transpose chunk m = 2r+h then holds moving columns j = 256r + 128h + q.
    x_v = x_in.rearrange("s (h q r) c -> s q r h c", h=2, q=128, r=4)
    # out[tb] flat = 65536*r + 256*c + q holds spike(c, n=4q+r); SBUF free j = 256r+q.
    out_v = out.rearrange("s (r ch cl) q -> s ch cl r q", r=4, ch=2, cl=128)
    w_v = w_in.rearrange("(oh p) c -> p oh c", oh=2, p=128)
    g_v = g_in.rearrange("(oh p) -> p oh", p=128)
    b_v = b_in.rearrange("(oh p) -> p oh", p=128)

    with ExitStack() as ctx:
        tc = ctx.enter_context(tile.TileContext(nc))
        consts = ctx.enter_context(tc.tile_pool(name="consts", bufs=1))
        natp = ctx.enter_context(tc.tile_pool(name="natp", bufs=BUFS["natp"]))
        xtps = ctx.enter_context(tc.tile_pool(name="xtps", bufs=BUFS["xtps"], space="PSUM"))
        xtsb = ctx.enter_context(tc.tile_pool(name="xtsb", bufs=BUFS["xtsb"]))
        yps = ctx.enter_context(tc.tile_pool(name="yps", bufs=BUFS["yps"], space="PSUM"))
        ybufp = ctx.enter_context(tc.tile_pool(name="ybufp", bufs=1))
        lifp = ctx.enter_context(tc.tile_pool(name="lifp", bufs=BUFS["lifp"]))
        vpool = ctx.enter_context(tc.tile_pool(name="vpool", bufs=BUFS["vpool"]))
        smallp = ctx.enter_context(tc.tile_pool(name="smallp", bufs=1))

        # identity from host (skips make_identity's gpsimd-prolog gate)
        ident_r = consts.tile([P, P], F32R)
        nc.sync.dma_start(out=ident_r, in_=id_in[:])

        # ---- constants: W^T tile (f32r), gamma/beta ----
        gam = consts.tile([P, 2], F32)
        nc.sync.dma_start(out=gam, in_=g_v)
        bet = consts.tile([P, 2], F32)
        nc.sync.dma_start(out=bet, in_=b_v)

        # warm up the CC rings early so the mid-kernel AllReduce is cheap
        if False and not SINGLE:
            warm_in, _ = tc.tile([P, 4], F32, space="DRAM", name="warm_in")
            warm_out, _ = tc.tile([P, 4], F32, space="DRAM", addr_space="Shared",
                                  name="warm_out")
            warm_sb = smallp.tile([P, 4], F32, name="warm_sb")
            nc.vector.memset(warm_sb, 0.0)
            nc.sync.dma_start(out=warm_in, in_=warm_sb)
            nc.gpsimd.collective_compute(
                "AllReduce", Alu.add,
                replica_groups=[list(range(N_CORES))],
                ins=[warm_in[:]], outs=[warm_out[:]],
            )

        # wt[:, ch, o] = W[o, ch*128+p] (f32r stationary tiles)
        wr_nat = lifp.tile([P, 2, C], F32R, name="wr_nat", tag="zs")
        nc.sync.dma_start(out=wr_nat, in_=w_v)
        wt = consts.tile([P, 2, C], F32R, name="wt_r")
        wtp = xtps.tile([P, 4, P], F32R, name="wtp", tag="xt_ps")
        for chh in range(2):
            for oh in range(2):
                nc.tensor.transpose(
                    wtp[:, chh * 2 + oh, :], wr_nat[:, oh, chh * P:(chh + 1) * P],
                    ident_r,
                )
        for chh in range(2):
            for oh in range(2):
                nc.scalar.copy(wt[:, chh, oh * P:(oh + 1) * P], wtp[:, chh * 2 + oh, :])

        # ---- phase 1: matmul all 16 slices, y -> SBUF, bn_stats ----
        y_sb = ybufp.tile([P, 2, SL, N], F32)          # 128KB/partition

        for _rep in range(REPEATS):
            _pipeline_body(nc, tc, consts, natp, xtps, xtsb, yps, lifp, vpool,
                           smallp, ident_r, wt, gam, bet, y_sb, x_v, out_v)

    _legalize_waits(nc)
    return nc


def _pipeline_body(nc, tc, consts, natp, xtps, xtsb, yps, lifp, vpool, smallp,
                   ident_r, wt, gam, bet, y_sb, x_v, out_v):
    if True:
        stat6 = smallp.tile([P, 2, 2 * SL, 6], F32, name="stat6")

        # Software pipeline: transposes for slice sl issue before the matmuls
        # of slice sl-1, so the PE never stalls on the PSUM->SBUF x^T copies.
        xt_pipe = {}
        for sl in range(SL + 1):
            if sl < SL:
                nat = natp.tile([P, 8, C], F32R, name="nat")
                if "dma_in" not in ABLATE:
                    nc.sync.dma_start(out=nat, in_=x_v[sl])
                xt_cur = []      # per chh: f32r moving operand [P, 1024]
                for chh in range(2):
                    xt_ps = xtps.tile([P, 1024], F32R, name="xt_ps", tag="xt_ps")
                    if "transpose" not in ABLATE:
                        for m in range(8):
                            nc.tensor.transpose(
                                xt_ps[:, m * P:(m + 1) * P],
                                nat[:, m, chh * P:(chh + 1) * P],
                                ident_r,
                            )
                    xt_r = xtsb.tile([P, 1024], F32R, name="xt_r", tag="xt_r")
                    if "xtcopy" not in ABLATE:
                        nc.scalar.copy(xt_r, xt_ps)
                    xt_cur.append(xt_r)
                xt_pipe[sl] = xt_cur
            if sl == 0:
                continue
            psl = sl - 1
            xt_sb = xt_pipe.pop(psl)
            for oh in range(2):
                yp = [yps.tile([P, 512], F32, name="yp") for _ in range(2)]
                if "mm" not in ABLATE:
                    # loop (chh, nsl): stationary reloads only on chh change
                    for chh in range(2):
                        for nsl in range(2):
                            nc.tensor.matmul(
                                yp[nsl],
                                wt[:, chh, oh * P:(oh + 1) * P],
                                xt_sb[chh][:, nsl * 512:(nsl + 1) * 512],
                                start=(chh == 0),
                                stop=(chh == 1),
                            )
                for nsl in range(2):
                    if "stats" not in ABLATE:
                        nc.vector.bn_stats(stat6[:, oh, psl * 2 + nsl, :], yp[nsl])
                    if "ycopy" not in ABLATE:
                        # split the PSUM drain between ACT and DVE
                        if (psl + oh) % 3 < 2:
                            nc.scalar.copy(
                                y_sb[:, oh, psl, nsl * 512:(nsl + 1) * 512], yp[nsl]
                            )
                        else:
                            nc.vector.tensor_copy(
                                y_sb[:, oh, psl, nsl * 512:(nsl + 1) * 512], yp[nsl]
                            )

        # ---- BN stats: per-core (sum, sumsq) -> AllReduce -> scale/shift ----
        if "stats" in ABLATE:
            return
        mv = smallp.tile([P, 2, 2], F32)
        for oh in range(2):
            nc.vector.bn_aggr(mv[:, oh, :], stat6[:, oh, :, :])
        ccs = smallp.tile([P, 4], F32)                 # [sum0, sum1, ssq0, ssq1]
        msq = smallp.tile([P, 2], F32)
        for oh in range(2):
            nc.vector.tensor_scalar(
                ccs[:, oh:oh + 1], mv[:, oh, 0:1], NS_CORE, None, Alu.mult
            )
            nc.vector.tensor_tensor(
                msq[:, oh:oh + 1], mv[:, oh, 0:1], mv[:, oh, 0:1], Alu.mult
            )
            nc.vector.scalar_tensor_tensor(
                ccs[:, 2 + oh:3 + oh], mv[:, oh, 1:2], NS_CORE, msq[:, oh:oh + 1],
                Alu.bypass, Alu.add,
            )
        # ccs[:, 2+oh] currently = var + mean^2 ; scale to sumsq
        nc.vector.tensor_scalar(ccs[:, 2:4], ccs[:, 2:4], NS_CORE, None, Alu.mult)

        cc_in, _ = tc.tile([P, 4], F32, space="DRAM", name="cc_in")
        cc_out, _ = tc.tile([P, 4], F32, space="DRAM", addr_space="Shared", name="cc_out")
        nc.sync.dma_start(out=cc_in, in_=ccs)
        if not SINGLE:
            nc.gpsimd.collective_compute(
                "AllReduce", Alu.add,
                replica_groups=[list(range(N_CORES))],
                ins=[cc_in[:]], outs=[cc_out[:]],
            )
        gst = smallp.tile([P, 4], F32)
        nc.sync.dma_start(out=gst, in_=cc_in if SINGLE else cc_out)

        mean_g = smallp.tile([P, 2], F32)
        nc.vector.tensor_scalar(mean_g, gst[:, 0:2], 1.0 / NS_TOT, None, Alu.mult)
        u = smallp.tile([P, 2], F32)                    # var + eps
        nc.vector.tensor_scalar(u, gst[:, 2:4], 1.0 / NS_TOT, None, Alu.mult)
        nc.vector.tensor_tensor(msq, mean_g, mean_g, Alu.mult)
        nc.vector.tensor_tensor(u, u, msq, Alu.subtract)
        nc.vector.tensor_scalar(u, u, BN_EPS, None, Alu.add)
        # rstd = 1/sqrt(u) with two Newton steps (ACT sqrt / DVE recip are approx)
        sq = smallp.tile([P, 2], F32)
        nc.scalar.sqrt(sq, u)
        r = smallp.tile([P, 2], F32)
        nc.vector.reciprocal(r, sq)
        t1 = smallp.tile([P, 2], F32)
        t2 = smallp.tile([P, 2], F32)
        for _ in range(2):
            nc.vector.tensor_tensor(t1, r, r, Alu.mult)
            nc.vector.tensor_tensor(t2, u, t1, Alu.mult)
            nc.vector.tensor_scalar(t2, t2, -0.5, 1.5, Alu.mult, Alu.add)
            nc.vector.tensor_tensor(r, r, t2, Alu.mult)
        # sc2 = 0.5*gamma*rstd ; sh2 = 0.5*beta - mean*sc2
        sc2 = smallp.tile([P, 2], F32)
        nc.vector.scalar_tensor_tensor(sc2, gam, 0.5, r, Alu.mult, Alu.mult)
        nc.vector.tensor_tensor(t1, mean_g, sc2, Alu.mult)
        sh2 = smallp.tile([P, 2], F32)
        nc.vector.scalar_tensor_tensor(sh2, bet, 0.5, t1, Alu.mult, Alu.subtract)

        # ---- phase 2: LIF ----
        # Engine split per (bl, t): BN-apply zh on ACT; v-update + reset on
        # DVE; spike threshold on DVE (6) / ACT via relu(sign(v-1)) (10).
        if "p2" in ABLATE:
            return
        neg1 = smallp.tile([P, 1], F32, name="neg1")
        nc.vector.memset(neg1, -1.0)
        for bl in range(B_LOC):
            v = vpool.tile([P, 2, N], F32, name="v")
            for t in range(T):
                sl = bl * 4 + t
                if t == 0:
                    for oh in range(2):
                        nc.scalar.activation(
                            v[:, oh, :], y_sb[:, oh, sl, :], ActF.Identity,
                            bias=sh2[:, oh:oh + 1], scale=sc2[:, oh:oh + 1],
                        )
                else:
                    zh = lifp.tile([P, 2, N], F32, name="zh", tag="zs")
                    for oh in range(2):
                        nc.scalar.activation(
                            zh[:, oh, :], y_sb[:, oh, sl, :], ActF.Identity,
                            bias=sh2[:, oh:oh + 1], scale=sc2[:, oh:oh + 1],
                        )
                    nc.vector.scalar_tensor_tensor(v, v, 0.5, zh, Alu.mult, Alu.add)
                s = lifp.tile([P, 2, N], F32, name="s", tag="zs")
                nc.vector.tensor_scalar(s, v, 1.0, None, Alu.is_ge)
                if "dma_out" not in ABLATE:
                    for oh in range(2):
                        nc.sync.dma_start(out=out_v[sl, oh], in_=s[:, oh, :])
                if t < 3:
                    nc.vector.scalar_tensor_tensor(v, v, 1.0, v, Alu.is_lt, Alu.mult)


_IDENT = np.eye(128, dtype=np.float32)

_nc_cache = None


def _get_nc():
    global _nc_cache
    if _nc_cache is None:
        _nc_cache = _build()
    return _nc_cache


def _tb_index(core, sl):
    bl, t = sl // T, sl % T
    return t * B + core * B_LOC + bl


def kernel(x, W, gamma, beta, _trace=False, _trace_kwargs=None):
    from concourse.bass_utils import run_bass_kernel_spmd

    x = np.ascontiguousarray(np.asarray(x, dtype=np.float32))
    W = np.ascontiguousarray(np.asarray(W, dtype=np.float32))
    gamma = np.ascontiguousarray(np.asarray(gamma, dtype=np.float32))
    beta = np.ascontiguousarray(np.asarray(beta, dtype=np.float32))

    nc = _get_nc()
    in_maps = []
    for k in range(N_CORES):
        idx = [_tb_index(k, sl) for sl in range(SL)]
        in_maps.append({
            "x": np.ascontiguousarray(x[idx]),
            "W": W, "gamma": gamma, "beta": beta,
            "ident": _IDENT,
        })
    kwargs = dict(_trace_kwargs or {})
    res = run_bass_kernel_spmd(
        nc, in_maps, core_ids=list(range(N_CORES)), trace=_trace, **kwargs
    )
    out = np.empty((T * B, N, C), dtype=np.float32)
    for k in range(N_CORES):
        ok = res.results[k]["out"]
        for sl in range(SL):
            out[_tb_index(k, sl)] = ok[sl]
    if _trace:
        return out, res
    return out


# revision 36
# speedup vs baseline: 1.0176x; 1.0176x over previous
"""Trainium2 Bass kernel for nn_Decoder1 (linear -> BatchNorm1d -> multistep LIF).

Reference computation (T=4, B=32, N=1024, C=256):
  y[tb,o,n]   = sum_c x[tb,n,c] * W[o,c]                      (TB=128 slices)
  z           = BN(y) over (tb, n) per channel o (training stats, eps=1e-5)
  LIF over t  : v' = (v + z_t)/2 ; s = (v' >= 1) ; v' *= (1-s)
  out[tb,n',c'] = spikes[tb].reshape(C,N).T   (row-major reinterpretation)

Sharding: data-parallel over B (batch) -> 4 b-values x 4 timesteps = 16
(N,C) slices per core. BN statistics via a tiny AllReduce of per-core
(sum, sumsq).

Matmul: single-term f32r (tolerance is rel-l2 2e-2 on binary spikes; f32r
rounding flips only a tiny number of near-threshold spikes).

Engine placement: PE transposes x + matmuls; ACT casts x^T f32->f32r and
applies BN in phase 2; DVE does bn_stats + LIF updates; Pool (gpsimd)
drains y PSUM->SBUF and shares LIF spike/reset work with DVE.

Layout trick: x rows are loaded in a permuted order (n = 4q+r -> column
j = 256r+q of the transposed moving operand) so the final spike tiles DMA
out to the (TB, C, N)->(TB, N, C) reinterpreted output with contiguous 1KB
runs on the DRAM side (no strided SBUF reads).
"""

import numpy as np
from contextlib import ExitStack

import concourse.bass as bass
import concourse.mybir as mybir
import concourse.tile as tile

F32 = mybir.dt.float32
F32R = mybir.dt.float32r
Alu = mybir.AluOpType
ActF = mybir.ActivationFunctionType

N_CORES = 8
T, B, N, C = 4, 32, 1024, 256
B_LOC = B // N_CORES            # 4 batch entries per core
SL = T * B_LOC                  # 16 (N,C) slices per core; sl = bl*4 + t
P = 128
NS_CORE = float(SL * N)         # BN samples per channel per core
NS_TOT = float(T * B * N)       # BN samples per channel globally
BN_EPS = 1e-5

_ctr = [0]
SINGLE = False   # test-only: skip the AllReduce (for single-core TimelineSim)
REPEATS = 1      # test-only: replicate the whole pipeline body for slope timing
ABLATE = set()   # test-only (sim): {"mm", "transpose", "ycopy", "stats", "p2", "dma_in", "dma_out"}
BUFS = {"natp": 4, "xtsb": 4, "yps": 4, "lifp": 3, "vpool": 2, "xtps": 2}


def _legalize_waits(nc, limit=1):
    """This walrus accepts very few semaphore waits per instruction (PE
    matmul: 1).  Hoist excess waits onto same-engine NoOps inserted just
    before the overloaded instruction (same engine => in-order => identical
    semantics)."""
    for f in nc.m.functions:
        for bb in f.blocks:
            new, dirty = [], False
            for ins in bb.instructions:
                si = ins.sync_info
                if si is not None and len(si.on_wait) > limit:
                    waits = list(si.on_wait)
                    for w in waits[:-limit]:
                        _ctr[0] += 1
                        no = mybir.InstNoOp(name=f"zwaitnop-{_ctr[0]}", ins=[], outs=[])
                        no.engine = ins.engine
                        no.sync_info = mybir.SyncInfo(on_wait=[w], on_update=[])
                        new.append(no)
                    ins.sync_info = mybir.SyncInfo(
                        on_wait=waits[-limit:], on_update=list(si.on_update)
                    )
                    dirty = True
                new.append(ins)
            if dirty:
                bb.instructions = new


def _build():
    nc = bass.Bass(num_devices=N_CORES)
    x_in = nc.declare_dram_parameter("x", [SL, N, C], F32R, isOutput=False)
    id_in = nc.declare_dram_parameter("ident", [P, P], F32R, isOutput=False)
    idh_in = nc.declare_dram_parameter("identh", [P, P], F32, isOutput=False)
    w_in = nc.declare_dram_parameter("W", [C, C], F32R, isOutput=False)
    g_in = nc.declare_dram_parameter("gamma", [C], F32, isOutput=False)
    b_in = nc.declare_dram_parameter("beta", [C], F32, isOutput=False)
    out = nc.declare_dram_parameter("out", [SL, N, C], F32, isOutput=True)

    # x rows n = h*512 + q*4 + r loaded so partition=q, free=(r,h,c):
    # transpose chunk m = 2r+h then holds moving columns j = 256r + 128h + q.
    x_v = x_in.rearrange("s (h q r) c -> s q h r c", h=2, q=128, r=4)
    # out[tb] flat = 65536*r + 256*c + q holds spike(c, n=4q+r); SBUF free j = 256r+q.
    out_v = out.rearrange("s (r ch cl) q -> s ch cl r q", r=4, ch=2, cl=128)
    w_v = w_in.rearrange("(oh p) c -> p oh c", oh=2, p=128)
    g_v = g_in.rearrange("(oh p) -> p oh", p=128)
    b_v = b_in.rearrange("(oh p) -> p oh", p=128)

    with ExitStack() as ctx:
        tc = ctx.enter_context(tile.TileContext(nc))
        consts = ctx.enter_context(tc.tile_pool(name="consts", bufs=1))
        natp = ctx.enter_context(tc.tile_pool(name="natp", bufs=BUFS["natp"]))
        xtps = ctx.enter_context(tc.tile_pool(name="xtps", bufs=BUFS["xtps"], space="PSUM"))
        xtsb = ctx.enter_context(tc.tile_pool(name="xtsb", bufs=BUFS["xtsb"]))
        yps = ctx.enter_context(tc.tile_pool(name="yps", bufs=BUFS["yps"], space="PSUM"))
        ybufp = ctx.enter_context(tc.tile_pool(name="ybufp", bufs=1))
        lifp = ctx.enter_context(tc.tile_pool(name="lifp", bufs=BUFS["lifp"]))
        vpool = ctx.enter_context(tc.tile_pool(name="vpool", bufs=BUFS["vpool"]))
        smallp = ctx.enter_context(tc.tile_pool(name="smallp", bufs=1))

        # identity from host (skips make_identity's gpsimd-prolog gate)
        ident_r = consts.tile([P, P], F32R)
        nc.sync.dma_start(out=ident_r, in_=id_in[:])
        identh = consts.tile([P, P], F32)
        nc.sync.dma_start(out=identh, in_=idh_in[:])

        # ---- constants: W^T tile (f32r), gamma/beta ----
        gam = consts.tile([P, 2], F32)
        nc.sync.dma_start(out=gam, in_=g_v)
        bet = consts.tile([P, 2], F32)
        nc.sync.dma_start(out=bet, in_=b_v)

        # warm up the CC rings early so the mid-kernel AllReduce is cheap
        if False and not SINGLE:
            warm_in, _ = tc.tile([P, 4], F32, space="DRAM", name="warm_in")
            warm_out, _ = tc.tile([P, 4], F32, space="DRAM", addr_space="Shared",
                                  name="warm_out")
            warm_sb = smallp.tile([P, 4], F32, name="warm_sb")
            nc.vector.memset(warm_sb, 0.0)
            nc.sync.dma_start(out=warm_in, in_=warm_sb)
            nc.gpsimd.collective_compute(
                "AllReduce", Alu.add,
                replica_groups=[list(range(N_CORES))],
                ins=[warm_in[:]], outs=[warm_out[:]],
            )

        # wt[:, ch, o] = W[o, ch*128+p] (f32r stationary tiles)
        wr_nat = lifp.tile([P, 2, C], F32R, name="wr_nat", tag="zs")
        nc.sync.dma_start(out=wr_nat, in_=w_v)
        wt = consts.tile([P, 2, C], F32R, name="wt_r")
        wtp = xtps.tile([P, 4, P], F32R, name="wtp", tag="xt_ps")
        for chh in range(2):
            for oh in range(2):
                nc.tensor.transpose(
                    wtp[:, chh * 2 + oh, :], wr_nat[:, oh, chh * P:(chh + 1) * P],
                    ident_r,
                )
        for chh in range(2):
            for oh in range(2):
                nc.scalar.copy(wt[:, chh, oh * P:(oh + 1) * P], wtp[:, chh * 2 + oh, :])

        # ---- phase 1: matmul all 16 slices, y -> SBUF, bn_stats ----
        y_sb = ybufp.tile([P, 2, SL, N], F32)          # 128KB/partition

        for _rep in range(REPEATS):
            _pipeline_body(nc, tc, consts, natp, xtps, xtsb, yps, lifp, vpool,
                           smallp, ident_r, identh, wt, gam, bet, y_sb, x_v, out_v)

    _legalize_waits(nc)
    return nc


def _pipeline_body(nc, tc, consts, natp, xtps, xtsb, yps, lifp, vpool, smallp,
                   ident_r, identh, wt, gam, bet, y_sb, x_v, out_v):
    if True:
        stat6 = smallp.tile([P, 2, 2 * SL, 6], F32, name="stat6")

        # Software pipeline: transposes for slice sl issue before the matmuls
        # of slice sl-1, so the PE never stalls on the PSUM->SBUF x^T copies.
        xt_pipe = {}
        for sl in range(SL + 1):
            if sl < SL:
                xt_ps = [xtps.tile([P, 1024], F32R, name="xt_ps", tag="xt_ps")
                         for _ in range(2)]
                for half in range(2):
                    nat = natp.tile([P, 2, 2, C], F32R, name="nat")
                    if "dma_in" not in ABLATE:
                        nc.sync.dma_start(
                            out=nat,
                            in_=x_v[sl, :, :, 2 * half:2 * half + 2],
                        )
                    if "transpose" not in ABLATE:
                        for mh in range(4):
                            m = half * 4 + mh
                            for chh in range(2):
                                nc.tensor.transpose(
                                    xt_ps[chh][:, m * P:(m + 1) * P],
                                    nat[:, mh % 2, mh // 2, chh * P:(chh + 1) * P],
                                    ident_r,
                                )
                xt_cur = []      # per chh: f32r moving operand [P, 1024]
                for chh in range(2):
                    xt_r = xtsb.tile([P, 1024], F32R, name="xt_r", tag="xt_r")
                    if "xtcopy" not in ABLATE:
                        nc.scalar.copy(xt_r, xt_ps[chh])
                    xt_cur.append(xt_r)
                xt_pipe[sl] = xt_cur
            if sl == 0:
                continue
            psl = sl - 1
            xt_sb = xt_pipe.pop(psl)
            for oh in range(2):
                yp = [yps.tile([P, 512], F32, name="yp") for _ in range(2)]
                if "mm" not in ABLATE:
                    # loop (chh, nsl): stationary reloads only on chh change
                    for chh in range(2):
                        for nsl in range(2):
                            nc.tensor.matmul(
                                yp[nsl],
                                wt[:, chh, oh * P:(oh + 1) * P],
                                xt_sb[chh][:, nsl * 512:(nsl + 1) * 512],
                                start=(chh == 0),
                                stop=(chh == 1),
                            )
                for nsl in range(2):
                    if "stats" not in ABLATE:
                        nc.vector.bn_stats(stat6[:, oh, psl * 2 + nsl, :], yp[nsl])
                    if "ycopy" not in ABLATE:
                        # split the PSUM drain between ACT and DVE
                        if (psl + oh) % 3 < 2:
                            nc.scalar.copy(
                                y_sb[:, oh, psl, nsl * 512:(nsl + 1) * 512], yp[nsl]
                            )
                        else:
                            nc.vector.tensor_copy(
                                y_sb[:, oh, psl, nsl * 512:(nsl + 1) * 512], yp[nsl]
                            )

        # ---- BN stats: per-core (sum, sumsq) -> AllReduce -> scale/shift ----
        if "stats" in ABLATE:
            return
        mv = smallp.tile([P, 2, 2], F32)
        for oh in range(2):
            nc.vector.bn_aggr(mv[:, oh, :], stat6[:, oh, :, :])
        ccs = smallp.tile([P, 4], F32)                 # [sum0, sum1, ssq0, ssq1]
        msq = smallp.tile([P, 2], F32)
        for oh in range(2):
            nc.vector.tensor_scalar(
                ccs[:, oh:oh + 1], mv[:, oh, 0:1], NS_CORE, None, Alu.mult
            )
            nc.vector.tensor_tensor(
                msq[:, oh:oh + 1], mv[:, oh, 0:1], mv[:, oh, 0:1], Alu.mult
            )
            nc.vector.scalar_tensor_tensor(
                ccs[:, 2 + oh:3 + oh], mv[:, oh, 1:2], NS_CORE, msq[:, oh:oh + 1],
                Alu.bypass, Alu.add,
            )
        # ccs[:, 2+oh] currently = var + mean^2 ; scale to sumsq
        nc.vector.tensor_scalar(ccs[:, 2:4], ccs[:, 2:4], NS_CORE, None, Alu.mult)

        cc_in, _ = tc.tile([P, 4], F32, space="DRAM", name="cc_in")
        cc_out, _ = tc.tile([N_CORES, P, 4], F32, space="DRAM", addr_space="Shared",
                            name="cc_out")
        nc.sync.dma_start(out=cc_in, in_=ccs)
        gst = smallp.tile([P, 4], F32)
        if not SINGLE:
            nc.gpsimd.collective_compute(
                "AllGather", Alu.bypass,
                replica_groups=[list(range(N_CORES))],
                ins=[cc_in[:]], outs=[cc_out[:]],
            )
            gall = smallp.tile([P, N_CORES, 4], F32)
            nc.sync.dma_start(
                out=gall, in_=cc_out.rearrange("k p f -> p k f")
            )
            # tree-reduce 8 -> 4 -> 2 -> 1 slots
            nc.vector.tensor_tensor(gall[:, 0:4, :], gall[:, 0:4, :],
                                    gall[:, 4:8, :], Alu.add)
            nc.vector.tensor_tensor(gall[:, 0:2, :], gall[:, 0:2, :],
                                    gall[:, 2:4, :], Alu.add)
            nc.vector.tensor_tensor(gst, gall[:, 0, :], gall[:, 1, :], Alu.add)
        else:
            nc.sync.dma_start(out=gst, in_=cc_in)

        mean_g = smallp.tile([P, 2], F32)
        nc.vector.tensor_scalar(mean_g, gst[:, 0:2], 1.0 / NS_TOT, None, Alu.mult)
        u = smallp.tile([P, 2], F32)                    # var + eps
        nc.vector.tensor_scalar(u, gst[:, 2:4], 1.0 / NS_TOT, None, Alu.mult)
        nc.vector.tensor_tensor(msq, mean_g, mean_g, Alu.mult)
        nc.vector.tensor_tensor(u, u, msq, Alu.subtract)
        nc.vector.tensor_scalar(u, u, BN_EPS, None, Alu.add)
        # rstd = 1/sqrt(u) with two Newton steps (ACT sqrt / DVE recip are approx)
        sq = smallp.tile([P, 2], F32)
        nc.scalar.sqrt(sq, u)
        r = smallp.tile([P, 2], F32)
        nc.vector.reciprocal(r, sq)
        t1 = smallp.tile([P, 2], F32)
        t2 = smallp.tile([P, 2], F32)
        for _ in range(2):
            nc.vector.tensor_tensor(t1, r, r, Alu.mult)
            nc.vector.tensor_tensor(t2, u, t1, Alu.mult)
            nc.vector.tensor_scalar(t2, t2, -0.5, 1.5, Alu.mult, Alu.add)
            nc.vector.tensor_tensor(r, r, t2, Alu.mult)
        # sc2 = 0.5*gamma*rstd ; sh2 = 0.5*beta - mean*sc2
        sc2 = smallp.tile([P, 2], F32)
        nc.vector.scalar_tensor_tensor(sc2, gam, 0.5, r, Alu.mult, Alu.mult)
        nc.vector.tensor_tensor(t1, mean_g, sc2, Alu.mult)
        sh2 = smallp.tile([P, 2], F32)
        nc.vector.scalar_tensor_tensor(sh2, bet, 0.5, t1, Alu.mult, Alu.subtract)

        # ---- phase 2: LIF ----
        # Engine split per (bl, t): BN-apply zh on ACT; v-update + reset on
        # DVE; spike threshold on DVE (6) / ACT via relu(sign(v-1)) (10).
        if "p2" in ABLATE:
            return
        neg1 = smallp.tile([P, 1], F32, name="neg1")
        nc.vector.memset(neg1, -1.0)
        for bl in range(B_LOC):
            v = vpool.tile([P, 2, N], F32, name="v")
            for t in range(T):
                sl = bl * 4 + t
                if t == 0:
                    for oh in range(2):
                        nc.scalar.activation(
                            v[:, oh, :], y_sb[:, oh, sl, :], ActF.Identity,
                            bias=sh2[:, oh:oh + 1], scale=sc2[:, oh:oh + 1],
                        )
                else:
                    zh = lifp.tile([P, 2, N], F32, name="zh", tag="zs")
                    for oh in range(2):
                        nc.scalar.activation(
                            zh[:, oh, :], y_sb[:, oh, sl, :], ActF.Identity,
                            bias=sh2[:, oh:oh + 1], scale=sc2[:, oh:oh + 1],
                        )
                    nc.vector.scalar_tensor_tensor(v, v, 0.5, zh, Alu.mult, Alu.add)
                s = lifp.tile([P, 2, N], F32, name="s", tag="zs")
                nc.vector.tensor_scalar(s, v, 1.0, None, Alu.is_ge)
                if "dma_out" not in ABLATE:
                    for oh in range(2):
                        nc.sync.dma_start(out=out_v[sl, oh], in_=s[:, oh, :])
                if t < 3:
                    nc.vector.scalar_tensor_tensor(v, v, 1.0, v, Alu.is_lt, Alu.mult)


_IDENT = np.eye(128, dtype=np.float32)
_IDENTH = 0.5 * np.eye(128, dtype=np.float32)

_nc_cache = None


def _get_nc():
    global _nc_cache
    if _nc_cache is None:
        _nc_cache = _build()
    return _nc_cache


def _tb_index(core, sl):
    bl, t = sl // T, sl % T
    return t * B + core * B_LOC + bl


def kernel(x, W, gamma, beta, _trace=False, _trace_kwargs=None):
    from concourse.bass_utils import run_bass_kernel_spmd

    x = np.ascontiguousarray(np.asarray(x, dtype=np.float32))
    W = np.ascontiguousarray(np.asarray(W, dtype=np.float32))
    gamma = np.ascontiguousarray(np.asarray(gamma, dtype=np.float32))
    beta = np.ascontiguousarray(np.asarray(beta, dtype=np.float32))

    nc = _get_nc()
    in_maps = []
    for k in range(N_CORES):
        idx = [_tb_index(k, sl) for sl in range(SL)]
        in_maps.append({
            "x": np.ascontiguousarray(x[idx]),
            "W": W, "gamma": gamma, "beta": beta,
            "ident": _IDENT, "identh": _IDENTH,
        })
    kwargs = dict(_trace_kwargs or {})
    res = run_bass_kernel_spmd(
        nc, in_maps, core_ids=list(range(N_CORES)), trace=_trace, **kwargs
    )
    out = np.empty((T * B, N, C), dtype=np.float32)
    for k in range(N_CORES):
        ok = res.results[k]["out"]
        for sl in range(SL):
            out[_tb_index(k, sl)] = ok[sl]
    if _trace:
        return out, res
    return out


# revision 37
# speedup vs baseline: 1.0441x; 1.0261x over previous
"""Trainium2 Bass kernel for nn_Decoder1 (linear -> BatchNorm1d -> multistep LIF).

Reference computation (T=4, B=32, N=1024, C=256):
  y[tb,o,n]   = sum_c x[tb,n,c] * W[o,c]                      (TB=128 slices)
  z           = BN(y) over (tb, n) per channel o (training stats, eps=1e-5)
  LIF over t  : v' = (v + z_t)/2 ; s = (v' >= 1) ; v' *= (1-s)
  out[tb,n',c'] = spikes[tb].reshape(C,N).T   (row-major reinterpretation)

Sharding: data-parallel over B (batch) -> 4 b-values x 4 timesteps = 16
(N,C) slices per core. BN statistics via a tiny AllReduce of per-core
(sum, sumsq).

Matmul: single-term f32r (tolerance is rel-l2 2e-2 on binary spikes; f32r
rounding flips only a tiny number of near-threshold spikes).

Engine placement: PE transposes x + matmuls; ACT casts x^T f32->f32r and
applies BN in phase 2; DVE does bn_stats + LIF updates; Pool (gpsimd)
drains y PSUM->SBUF and shares LIF spike/reset work with DVE.

Layout trick: x rows are loaded in a permuted order (n = 4q+r -> column
j = 256r+q of the transposed moving operand) so the final spike tiles DMA
out to the (TB, C, N)->(TB, N, C) reinterpreted output with contiguous 1KB
runs on the DRAM side (no strided SBUF reads).
"""

import numpy as np
from contextlib import ExitStack

import concourse.bass as bass
import concourse.mybir as mybir
import concourse.tile as tile

F32 = mybir.dt.float32
F32R = mybir.dt.float32r
Alu = mybir.AluOpType
ActF = mybir.ActivationFunctionType

N_CORES = 8
T, B, N, C = 4, 32, 1024, 256
B_LOC = B // N_CORES            # 4 batch entries per core
SL = T * B_LOC                  # 16 (N,C) slices per core; sl = bl*4 + t
P = 128
NS_CORE = float(SL * N)         # BN samples per channel per core
NS_TOT = float(T * B * N)       # BN samples per channel globally
BN_EPS = 1e-5

_ctr = [0]
SINGLE = False   # test-only: skip the AllReduce (for single-core TimelineSim)
REPEATS = 1      # test-only: replicate the whole pipeline body for slope timing
ABLATE = set()   # test-only (sim): {"mm", "transpose", "ycopy", "stats", "p2", "dma_in", "dma_out"}
BUFS = {"natp": 4, "xtsb": 4, "yps": 4, "lifp": 3, "vpool": 2, "xtps": 2}


def _legalize_waits(nc, limit=1):
    """This walrus accepts very few semaphore waits per instruction (PE
    matmul: 1).  Hoist excess waits onto same-engine NoOps inserted just
    before the overloaded instruction (same engine => in-order => identical
    semantics)."""
    for f in nc.m.functions:
        for bb in f.blocks:
            new, dirty = [], False
            for ins in bb.instructions:
                si = ins.sync_info
                if si is not None and len(si.on_wait) > limit:
                    waits = list(si.on_wait)
                    for w in waits[:-limit]:
                        _ctr[0] += 1
                        no = mybir.InstNoOp(name=f"zwaitnop-{_ctr[0]}", ins=[], outs=[])
                        no.engine = ins.engine
                        no.sync_info = mybir.SyncInfo(on_wait=[w], on_update=[])
                        new.append(no)
                    ins.sync_info = mybir.SyncInfo(
                        on_wait=waits[-limit:], on_update=list(si.on_update)
                    )
                    dirty = True
                new.append(ins)
            if dirty:
                bb.instructions = new


def _build():
    nc = bass.Bass(num_devices=N_CORES)
    x_in = nc.declare_dram_parameter("x", [SL, N, C], F32R, isOutput=False)
    id_in = nc.declare_dram_parameter("ident", [P, P], F32R, isOutput=False)
    idh_in = nc.declare_dram_parameter("identh", [P, P], F32, isOutput=False)
    w_in = nc.declare_dram_parameter("W", [C, C], F32R, isOutput=False)
    g_in = nc.declare_dram_parameter("gamma", [C], F32, isOutput=False)
    b_in = nc.declare_dram_parameter("beta", [C], F32, isOutput=False)
    out = nc.declare_dram_parameter("out", [SL, N, C], F32, isOutput=True)

    # x rows n = h*512 + q*4 + r loaded so partition=q, free=(r,h,c):
    # transpose chunk m = 2r+h then holds moving columns j = 256r + 128h + q.
    x_v = x_in.rearrange("s (h q r) c -> s q h r c", h=2, q=128, r=4)
    # out[tb] flat = 65536*r + 256*c + q holds spike(c, n=4q+r); SBUF free j = 256r+q.
    out_v = out.rearrange("s (r ch cl) q -> s ch cl r q", r=4, ch=2, cl=128)
    w_v = w_in.rearrange("(oh p) c -> p oh c", oh=2, p=128)
    g_v = g_in.rearrange("(oh p) -> p oh", p=128)
    b_v = b_in.rearrange("(oh p) -> p oh", p=128)

    with ExitStack() as ctx:
        tc = ctx.enter_context(tile.TileContext(nc))
        consts = ctx.enter_context(tc.tile_pool(name="consts", bufs=1))
        natp = ctx.enter_context(tc.tile_pool(name="natp", bufs=BUFS["natp"]))
        xtps = ctx.enter_context(tc.tile_pool(name="xtps", bufs=BUFS["xtps"], space="PSUM"))
        xtsb = ctx.enter_context(tc.tile_pool(name="xtsb", bufs=BUFS["xtsb"]))
        yps = ctx.enter_context(tc.tile_pool(name="yps", bufs=BUFS["yps"], space="PSUM"))
        ybufp = ctx.enter_context(tc.tile_pool(name="ybufp", bufs=1))
        lifp = ctx.enter_context(tc.tile_pool(name="lifp", bufs=BUFS["lifp"]))
        vpool = ctx.enter_context(tc.tile_pool(name="vpool", bufs=BUFS["vpool"]))
        smallp = ctx.enter_context(tc.tile_pool(name="smallp", bufs=1))

        # identity from host (skips make_identity's gpsimd-prolog gate)
        ident_r = consts.tile([P, P], F32R)
        nc.sync.dma_start(out=ident_r, in_=id_in[:])
        identh = consts.tile([P, P], F32)
        nc.sync.dma_start(out=identh, in_=idh_in[:])

        # ---- constants: W^T tile (f32r), gamma/beta ----
        gam = consts.tile([P, 2], F32)
        nc.sync.dma_start(out=gam, in_=g_v)
        bet = consts.tile([P, 2], F32)
        nc.sync.dma_start(out=bet, in_=b_v)

        # warm up the CC rings early so the mid-kernel AllReduce is cheap
        if False and not SINGLE:
            warm_in, _ = tc.tile([P, 4], F32, space="DRAM", name="warm_in")
            warm_out, _ = tc.tile([P, 4], F32, space="DRAM", addr_space="Shared",
                                  name="warm_out")
            warm_sb = smallp.tile([P, 4], F32, name="warm_sb")
            nc.vector.memset(warm_sb, 0.0)
            nc.sync.dma_start(out=warm_in, in_=warm_sb)
            nc.gpsimd.collective_compute(
                "AllReduce", Alu.add,
                replica_groups=[list(range(N_CORES))],
                ins=[warm_in[:]], outs=[warm_out[:]],
            )

        # wt[:, ch, o] = W[o, ch*128+p] (f32r stationary tiles)
        wr_nat = lifp.tile([P, 2, C], F32R, name="wr_nat", tag="zs")
        nc.sync.dma_start(out=wr_nat, in_=w_v)
        wt = consts.tile([P, 2, C], F32R, name="wt_r")
        wtp = xtps.tile([P, 4, P], F32R, name="wtp", tag="xt_ps")
        for chh in range(2):
            for oh in range(2):
                nc.tensor.transpose(
                    wtp[:, chh * 2 + oh, :], wr_nat[:, oh, chh * P:(chh + 1) * P],
                    ident_r,
                )
        for chh in range(2):
            for oh in range(2):
                nc.scalar.copy(wt[:, chh, oh * P:(oh + 1) * P], wtp[:, chh * 2 + oh, :])

        # ---- phase 1: matmul all 16 slices, y -> SBUF, bn_stats ----
        y_sb = ybufp.tile([P, 2, SL, N], F32)          # 128KB/partition

        for _rep in range(REPEATS):
            _pipeline_body(nc, tc, consts, natp, xtps, xtsb, yps, lifp, vpool,
                           smallp, ident_r, identh, wt, gam, bet, y_sb, x_v, out_v)

    _legalize_waits(nc)
    return nc


def _pipeline_body(nc, tc, consts, natp, xtps, xtsb, yps, lifp, vpool, smallp,
                   ident_r, identh, wt, gam, bet, y_sb, x_v, out_v):
    if True:
        stat6 = smallp.tile([P, 2, 2 * SL, 6], F32, name="stat6")

        # Software pipeline: transposes for slice sl issue before the matmuls
        # of slice sl-1, so the PE never stalls on the PSUM->SBUF x^T copies.
        xt_pipe = {}
        for sl in range(SL + 1):
            if sl < SL:
                xt_ps = [xtps.tile([P, 1024], F32R, name="xt_ps", tag="xt_ps")
                         for _ in range(2)]
                for half in range(2):
                    nat = natp.tile([P, 2, 2, C], F32R, name="nat")
                    if "dma_in" not in ABLATE:
                        nc.sync.dma_start(
                            out=nat,
                            in_=x_v[sl, :, :, 2 * half:2 * half + 2],
                        )
                    if "transpose" not in ABLATE:
                        for mh in range(4):
                            m = half * 4 + mh
                            for chh in range(2):
                                nc.tensor.transpose(
                                    xt_ps[chh][:, m * P:(m + 1) * P],
                                    nat[:, mh % 2, mh // 2, chh * P:(chh + 1) * P],
                                    ident_r,
                                )
                xt_cur = []      # per chh: f32r moving operand [P, 1024]
                for chh in range(2):
                    xt_r = xtsb.tile([P, 1024], F32R, name="xt_r", tag="xt_r")
                    if "xtcopy" not in ABLATE:
                        nc.scalar.copy(xt_r, xt_ps[chh])
                    xt_cur.append(xt_r)
                xt_pipe[sl] = xt_cur
            if sl == 0:
                continue
            psl = sl - 1
            xt_sb = xt_pipe.pop(psl)
            for oh in range(2):
                yp = [yps.tile([P, 512], F32, name="yp") for _ in range(2)]
                if "mm" not in ABLATE:
                    # loop (chh, nsl): stationary reloads only on chh change
                    for chh in range(2):
                        for nsl in range(2):
                            nc.tensor.matmul(
                                yp[nsl],
                                wt[:, chh, oh * P:(oh + 1) * P],
                                xt_sb[chh][:, nsl * 512:(nsl + 1) * 512],
                                start=(chh == 0),
                                stop=(chh == 1),
                            )
                for nsl in range(2):
                    if "stats" not in ABLATE:
                        nc.vector.bn_stats(stat6[:, oh, psl * 2 + nsl, :], yp[nsl])
                    if "ycopy" not in ABLATE:
                        # split the PSUM drain between ACT and DVE
                        if (psl + oh) % 3 < 2:
                            nc.scalar.copy(
                                y_sb[:, oh, psl, nsl * 512:(nsl + 1) * 512], yp[nsl]
                            )
                        else:
                            nc.vector.tensor_copy(
                                y_sb[:, oh, psl, nsl * 512:(nsl + 1) * 512], yp[nsl]
                            )

        # ---- BN stats: per-core (sum, sumsq) -> AllReduce -> scale/shift ----
        if "stats" in ABLATE:
            return
        mv = smallp.tile([P, 2, 2], F32)
        for oh in range(2):
            nc.vector.bn_aggr(mv[:, oh, :], stat6[:, oh, :, :])
        ccs = smallp.tile([P, 4], F32)                 # [sum0, sum1, ssq0, ssq1]
        msq = smallp.tile([P, 2], F32)
        for oh in range(2):
            nc.vector.tensor_scalar(
                ccs[:, oh:oh + 1], mv[:, oh, 0:1], NS_CORE, None, Alu.mult
            )
            nc.vector.tensor_tensor(
                msq[:, oh:oh + 1], mv[:, oh, 0:1], mv[:, oh, 0:1], Alu.mult
            )
            nc.vector.scalar_tensor_tensor(
                ccs[:, 2 + oh:3 + oh], mv[:, oh, 1:2], NS_CORE, msq[:, oh:oh + 1],
                Alu.bypass, Alu.add,
            )
        # ccs[:, 2+oh] currently = var + mean^2 ; scale to sumsq
        nc.vector.tensor_scalar(ccs[:, 2:4], ccs[:, 2:4], NS_CORE, None, Alu.mult)

        cc_in, _ = tc.tile([P, 4], F32, space="DRAM", name="cc_in")
        cc_out, _ = tc.tile([P, 4], F32, space="DRAM", addr_space="Shared", name="cc_out")
        nc.sync.dma_start(out=cc_in, in_=ccs)
        if not SINGLE:
            nc.gpsimd.collective_compute(
                "AllReduce", Alu.add,
                replica_groups=[list(range(N_CORES))],
                ins=[cc_in[:]], outs=[cc_out[:]],
            )
        gst = smallp.tile([P, 4], F32)
        nc.sync.dma_start(out=gst, in_=cc_in if SINGLE else cc_out)

        mean_g = smallp.tile([P, 2], F32)
        nc.vector.tensor_scalar(mean_g, gst[:, 0:2], 1.0 / NS_TOT, None, Alu.mult)
        u = smallp.tile([P, 2], F32)                    # var + eps
        nc.vector.tensor_scalar(u, gst[:, 2:4], 1.0 / NS_TOT, None, Alu.mult)
        nc.vector.tensor_tensor(msq, mean_g, mean_g, Alu.mult)
        nc.vector.tensor_tensor(u, u, msq, Alu.subtract)
        nc.vector.tensor_scalar(u, u, BN_EPS, None, Alu.add)
        # rstd = 1/sqrt(u) with two Newton steps (ACT sqrt / DVE recip are approx)
        sq = smallp.tile([P, 2], F32)
        nc.scalar.sqrt(sq, u)
        r = smallp.tile([P, 2], F32)
        nc.vector.reciprocal(r, sq)
        t1 = smallp.tile([P, 2], F32)
        t2 = smallp.tile([P, 2], F32)
        for _ in range(2):
            nc.vector.tensor_tensor(t1, r, r, Alu.mult)
            nc.vector.tensor_tensor(t2, u, t1, Alu.mult)
            nc.vector.tensor_scalar(t2, t2, -0.5, 1.5, Alu.mult, Alu.add)
            nc.vector.tensor_tensor(r, r, t2, Alu.mult)
        # sc2 = 0.5*gamma*rstd ; sh2 = 0.5*beta - mean*sc2
        sc2 = smallp.tile([P, 2], F32)
        nc.vector.scalar_tensor_tensor(sc2, gam, 0.5, r, Alu.mult, Alu.mult)
        nc.vector.tensor_tensor(t1, mean_g, sc2, Alu.mult)
        sh2 = smallp.tile([P, 2], F32)
        nc.vector.scalar_tensor_tensor(sh2, bet, 0.5, t1, Alu.mult, Alu.subtract)

        # ---- phase 2: LIF ----
        # Engine split per (bl, t): BN-apply zh on ACT; v-update + reset on
        # DVE; spike threshold on DVE (6) / ACT via relu(sign(v-1)) (10).
        if "p2" in ABLATE:
            return
        neg1 = smallp.tile([P, 1], F32, name="neg1")
        nc.vector.memset(neg1, -1.0)
        for bl in range(B_LOC):
            v = vpool.tile([P, 2, N], F32, name="v")
            for t in range(T):
                sl = bl * 4 + t
                if t == 0:
                    for oh in range(2):
                        nc.scalar.activation(
                            v[:, oh, :], y_sb[:, oh, sl, :], ActF.Identity,
                            bias=sh2[:, oh:oh + 1], scale=sc2[:, oh:oh + 1],
                        )
                else:
                    zh = lifp.tile([P, 2, N], F32, name="zh", tag="zs")
                    for oh in range(2):
                        nc.scalar.activation(
                            zh[:, oh, :], y_sb[:, oh, sl, :], ActF.Identity,
                            bias=sh2[:, oh:oh + 1], scale=sc2[:, oh:oh + 1],
                        )
                    nc.vector.scalar_tensor_tensor(v, v, 0.5, zh, Alu.mult, Alu.add)
                s = lifp.tile([P, 2, N], F32, name="s", tag="zs")
                nc.vector.tensor_scalar(s, v, 1.0, None, Alu.is_ge)
                if "dma_out" not in ABLATE:
                    for oh in range(2):
                        nc.sync.dma_start(out=out_v[sl, oh], in_=s[:, oh, :])
                if t < 3:
                    nc.vector.scalar_tensor_tensor(v, v, 1.0, v, Alu.is_lt, Alu.mult)


_IDENT = np.eye(128, dtype=np.float32)
_IDENTH = 0.5 * np.eye(128, dtype=np.float32)

_nc_cache = None


def _get_nc():
    global _nc_cache
    if _nc_cache is None:
        _nc_cache = _build()
    return _nc_cache


def _tb_index(core, sl):
    bl, t = sl // T, sl % T
    return t * B + core * B_LOC + bl


def kernel(x, W, gamma, beta, _trace=False, _trace_kwargs=None):
    from concourse.bass_utils import run_bass_kernel_spmd

    x = np.ascontiguousarray(np.asarray(x, dtype=np.float32))
    W = np.ascontiguousarray(np.asarray(W, dtype=np.float32))
    gamma = np.ascontiguousarray(np.asarray(gamma, dtype=np.float32))
    beta = np.ascontiguousarray(np.asarray(beta, dtype=np.float32))

    nc = _get_nc()
    in_maps = []
    for k in range(N_CORES):
        idx = [_tb_index(k, sl) for sl in range(SL)]
        in_maps.append({
            "x": np.ascontiguousarray(x[idx]),
            "W": W, "gamma": gamma, "beta": beta,
            "ident": _IDENT, "identh": _IDENTH,
        })
    kwargs = dict(_trace_kwargs or {})
    res = run_bass_kernel_spmd(
        nc, in_maps, core_ids=list(range(N_CORES)), trace=_trace, **kwargs
    )
    out = np.empty((T * B, N, C), dtype=np.float32)
    for k in range(N_CORES):
        ok = res.results[k]["out"]
        for sl in range(SL):
            out[_tb_index(k, sl)] = ok[sl]
    if _trace:
        return out, res
    return out
